# revision 1
# baseline (speedup 1.0000x reference)
"""COOTensorProduct kernel for 8 Trainium2 NeuronCores.

Math: out[b, h] = sum_{i,j} cb[h, i*64+j] * in1[b, i] * in2[b, j]
with in1/in2 [4096, 64], cb [4096, 4096] (a Clebsch-Gordan / Wigner-3j
coupling matrix for irreps '4x0e+4x1o+4x2e+4x3o' x same -> all l3).

cb is 0.1% dense but perfectly block-structured: for each (l1, l2) pair
of irrep types the coupling is a square (2l1+1)(2l2+1) x (2l1+1)(2l2+1)
matrix (stacked l3 blocks), identical across the 4x4 multiplicity copies
(u, v). The 16 pair matrices have sizes {1,3,3,5,5,7,7,9,15,15,21,21,
25,35,35,49} which pack block-diagonally into exactly two 128x128
stationary matrices (49+35+35+9 = 128 and the rest = 128).

Per core (512 batch rows):
  rhs[S][u,v]  = in1T_gathered[S,u] * in2T_gathered[S,v]   (elementwise,
                 [128 partitions = (pair,m1,m2) rows, 512 free = batch])
  psum[S][u,v] = W_S.T @ rhs        (one 128x128x512 matmul)
so the whole problem is 32 elementwise mults + 32 matmuls per core.

Host does the (static, index-only) gathers/permutes; device does all
FLOPs. Output comes back as [4096 permuted rows, 512 batch] per core and
is un-permuted/transposed on host.
"""

import json
import numpy as np

# ---------------------------------------------------------------- problem
B = 4096
DIM = 64
NCORES = 8
BPC = B // NCORES  # 512 batch rows per core
LMAX = 3
NMULT = 4  # multiplicity of each l in '4x0e+4x1o+4x2e+4x3o'
LS = [l for l in range(LMAX + 1) for _ in range(NMULT)]

# block-diagonal packing of the 16 (l1,l2) pair matrices into 2 stationaries
PAIRS_A = [(3, 3), (3, 2), (2, 3), (1, 1)]
PAIRS_B = [(2, 2), (1, 3), (3, 1), (1, 2), (2, 1), (0, 3), (3, 0),
           (0, 2), (2, 0), (0, 1), (1, 0), (0, 0)]

_decomp_cache = None
_nc_cache = None


def _col_start(l, u):
    return sum((2 * ll + 1) * NMULT for ll in range(l)) + u * (2 * l + 1)


def _build_decomp():
    """Index bookkeeping only (no numerics): which cb entries form the two
    stationary matrices, which in1/in2 columns feed each partition row,
    and which output row h each psum row maps to."""
    global _decomp_cache
    if _decomp_cache is not None:
        return _decomp_cache

    # replicate build_cb_matrix's row layout
    layout = {}
    idx1 = 0
    for l1 in LS:
        idx2 = 0
        for l2 in LS:
            for l3 in range(abs(l1 - l2), l1 + l2 + 1):
                layout.setdefault(l3, []).append((l1, l2, idx1 * DIM + idx2))
            idx2 += 2 * l2 + 1
        idx1 += 2 * l1 + 1
    entry_row = {}
    row = 0
    for l3 in sorted(layout):
        for (l1, l2, co) in sorted(layout[l3], key=lambda x: x[0] * LMAX + x[1]):
            entry_row[(l3, co)] = row
            row += 2 * l3 + 1
    assert row == B

    groups = []
    for pairs in (PAIRS_A, PAIRS_B):
        assert sum((2 * a + 1) * (2 * b + 1) for a, b in pairs) == 128
        c1 = np.zeros((NMULT, 128), dtype=np.int64)
        c2 = np.zeros((NMULT, 128), dtype=np.int64)
        h_of = np.zeros((NMULT, NMULT, 128), dtype=np.int64)
        w_k, w_m, w_h, w_c = [], [], [], []  # W[k,m] = cb[h, c]
        off = 0
        for (l1, l2) in pairs:
            n1, n2 = 2 * l1 + 1, 2 * l2 + 1
            kp = n1 * n2
            kk = np.arange(kp)
            m1, m2 = kk // n2, kk % n2
            for u in range(NMULT):
                c1[u, off:off + kp] = _col_start(l1, u) + m1
            for v in range(NMULT):
                c2[v, off:off + kp] = _col_start(l2, v) + m2
            mm = 0
            for l3 in range(abs(l1 - l2), l1 + l2 + 1):
                n3 = 2 * l3 + 1
                h0 = entry_row[(l3, _col_start(l1, 0) * DIM + _col_start(l2, 0))]
                km, m3m = np.meshgrid(kk, np.arange(n3), indexing="ij")
                w_k.append((off + km).ravel())
                w_m.append((off + mm + m3m).ravel())
                w_h.append((h0 + m3m).ravel())
                w_c.append(((_col_start(l1, 0) + m1[km.ravel()]) * DIM
                            + (_col_start(l2, 0) + m2[km.ravel()])))
                for u in range(NMULT):
                    for v in range(NMULT):
                        h = entry_row[(l3, _col_start(l1, u) * DIM + _col_start(l2, v))]
                        h_of[u, v, off + mm:off + mm + n3] = np.arange(h, h + n3)
                mm += n3
            off += kp
        groups.append({
            "c1": c1, "c2": c2, "h_of": h_of,
            "w_k": np.concatenate(w_k), "w_m": np.concatenate(w_m),
            "w_h": np.concatenate(w_h), "w_c": np.concatenate(w_c),
        })

    # global output row -> h map: tile t = S*16 + u*4 + v holds rows
    # t*128 + mm  ->  h_of[S][u, v, mm]
    hglob = np.zeros(32 * 128, dtype=np.int64)
    for s, g in enumerate(groups):
        for u in range(NMULT):
            for v in range(NMULT):
                t = s * 16 + u * 4 + v
                hglob[t * 128:(t + 1) * 128] = g["h_of"][u, v]
    _decomp_cache = (groups, hglob)
    return _decomp_cache


def _split_waits(bir_bytes):
    """This container's walrus build rejects >1 sync-wait per instruction
    ("Too many sync wait commands"). Hoist extra waits onto standalone
    EventSemaphore instructions on the same engine (same lowering raw
    bass wait_ge uses)."""
    bir = json.loads(bir_bytes)
    n = 0
    for fn in bir["functions"]:
        for blk in fn["blocks"]:
            out = []
            for inst in blk["instructions"]:
                si = inst.get("sync_info")
                waits = (si or {}).get("on_wait") or []
                if len(waits) > 1:
                    for w in waits[:-1]:
                        n += 1
                        out.append({
                            "debug": inst.get("debug", 0),
                            "engine": inst["engine"],
                            "ins": [], "outs": [],
                            "name": f"I-wsplit-{n}",
                            "opcode": "EventSemaphore",
                            "sync_info": {"on_update": [], "on_wait": [w]},
                        })
                    si["on_wait"] = [waits[-1]]
                out.append(inst)
            blk["instructions"] = out
    return json.dumps(bir).encode()


def _build_nc():
    """Bass program, identical on all 8 cores (SPMD; per-core data differs).

    Per core: 16 pre-gathered input tiles [128, 512] (partition = the
    (pair, m1, m2) rows of one packed stationary group, free = batch),
    32 elementwise products (DVE), 32 [128x128x512] matmuls against the
    two block-diagonal CG stationaries (PE), 32 PSUM->SBUF copies (ACT),
    32 output DMAs. Measured fastest of all variants profiled (fp32
    throughout; float32r / PE-side gather / fused-mult / packed-DMA /
    raw-pre-Tile-DMA variants all traced equal, slower, or incorrect).
    The kernel sits simultaneously near the fp32 PE floor (4 cyc/row),
    the DMA fabric floor (12.6 MB/core), and the DVE/ACT 1x elementwise
    floors.
    """
    global _nc_cache
    if _nc_cache is not None:
        return _nc_cache
    import concourse.bass as bass
    import concourse.mybir as mybir
    from concourse.tile import TileContext

    f32 = mybir.dt.float32
    nc = bass.Bass()
    w = nc.dram_tensor("w", [2, 128, 128], f32, kind="ExternalInput")
    g1 = nc.dram_tensor("g1", [8, 128, BPC], f32, kind="ExternalInput")
    g2 = nc.dram_tensor("g2", [8, 128, BPC], f32, kind="ExternalInput")
    o = nc.dram_tensor("o", [32, 128, BPC], f32, kind="ExternalOutput")

    with TileContext(nc) as tc:
        with (
            tc.tile_pool(name="wpool", bufs=1) as wpool,
            tc.tile_pool(name="gpool", bufs=1) as gpool,
            tc.tile_pool(name="rhspool", bufs=6) as rhspool,
            tc.tile_pool(name="psum", bufs=8, space="PSUM") as psumpool,
            tc.tile_pool(name="opool", bufs=8) as opool,
        ):
            wt = []
            for s in range(2):
                t = wpool.tile([128, 128], f32, tag=f"w{s}", name=f"w{s}")
                nc.sync.dma_start(out=t, in_=w[s, :, :])
                wt.append(t)
            g1t, g2t = [], []
            for i in range(8):
                t = gpool.tile([128, BPC], f32, tag=f"g1_{i}", name=f"g1_{i}")
                nc.sync.dma_start(out=t, in_=g1[i, :, :])
                g1t.append(t)
                t = gpool.tile([128, BPC], f32, tag=f"g2_{i}", name=f"g2_{i}")
                nc.sync.dma_start(out=t, in_=g2[i, :, :])
                g2t.append(t)

            for s in range(2):
                for u in range(NMULT):
                    for v in range(NMULT):
                        t = s * 16 + u * 4 + v
                        rhs = rhspool.tile([128, BPC], f32, tag="rhs")
                        nc.vector.tensor_mul(
                            out=rhs, in0=g1t[s * 4 + u], in1=g2t[s * 4 + v])
                        ps = psumpool.tile([128, BPC], f32, tag="ps")
                        nc.tensor.matmul(ps, wt[s], rhs, start=True, stop=True)
                        ot = opool.tile([128, BPC], f32, tag="ot")
                        if t >= 28:
                            # tail copies on DVE (its mults are done by now)
                            nc.vector.tensor_copy(out=ot, in_=ps)
                        else:
                            nc.scalar.copy(out=ot, in_=ps)
                        nc.sync.dma_start(out=o[t, :, :], in_=ot)

    orig = nc.to_json_bytes
    nc.to_json_bytes = lambda: _split_waits(orig())
    _nc_cache = nc
    return nc


def kernel(in1, in2, cb, _want_stats=False):
    from concourse.bass_utils import run_bass_kernel_spmd

    in1 = np.ascontiguousarray(np.asarray(in1, dtype=np.float32))
    in2 = np.ascontiguousarray(np.asarray(in2, dtype=np.float32))
    cb = np.asarray(cb, dtype=np.float32)
    groups, hglob = _build_decomp()

    # stationaries extracted straight from cb (no wigner math needed)
    wmat = np.zeros((2, 128, 128), dtype=np.float32)
    for s, g in enumerate(groups):
        wmat[s][g["w_k"], g["w_m"]] = cb[g["w_h"], g["w_c"]]

    in_maps = []
    for c in range(NCORES):
        sl = slice(c * BPC, (c + 1) * BPC)
        b1, b2 = in1[sl], in2[sl]
        gg1 = np.empty((8, 128, BPC), dtype=np.float32)
        gg2 = np.empty((8, 128, BPC), dtype=np.float32)
        for s, g in enumerate(groups):
            for u in range(NMULT):
                gg1[s * 4 + u] = b1.T[g["c1"][u]]
                gg2[s * 4 + u] = b2.T[g["c2"][u]]
        in_maps.append({"w": wmat, "g1": gg1, "g2": gg2})

    nc = _build_nc()
    import os
    trace = bool(int(os.environ.get("KERNEL_TRACE", "0")))
    res = run_bass_kernel_spmd(nc, in_maps, core_ids=list(range(NCORES)),
                               trace=trace)

    # [4096 permuted rows, 4096 batch]
    full = np.concatenate(
        [r["o"].reshape(32 * 128, BPC) for r in res.results], axis=1)
    out = np.empty((B, B), dtype=np.float32)
    out[:, hglob] = full.T
    if _want_stats:
        return out, res
    return out


if __name__ == "__main__":
    rng = np.random.default_rng(0)
    a = rng.standard_normal((B, DIM)).astype(np.float32)
    b = rng.standard_normal((B, DIM)).astype(np.float32)
    cb = np.load("/tmp/cb.npy")
    out = kernel(a, b, cb)
    outer = np.einsum("bi,bj->bij", a, b).reshape(B, -1)
    exp = outer @ cb.T
    print("rel err:", np.linalg.norm(out - exp) / np.linalg.norm(exp))



# revision 4
# speedup vs baseline: 1.3104x; 1.3104x over previous
"""COOTensorProduct kernel for 8 Trainium2 NeuronCores — bf16 edition.

Math: out[b, h] = sum_{i,j} cb[h, i*64+j] * in1[b, i] * in2[b, j]
with in1/in2 [4096, 64], cb [4096, 4096] (Clebsch-Gordan coupling for
irreps '4x0e+4x1o+4x2e+4x3o' x same -> all l3).

cb is block-structured: the 16 (l1,l2) pair couplings pack into two
128x128 block-diagonal stationaries. Per core (512 batch rows):
  rhs[s,u,v] = g1[s,u] * g2[s,v]            (DVE, bf16, 2x mode)
  psum[s,u,v] = W_s.T @ rhs                 (PE, bf16, 1 cyc/row)
  out tile = bf16(psum)                     (ACT/Pool copies)
The rel-err budget (2e-2) dwarfs bf16 rounding (~0.5%), so all traffic
and compute run in bf16; psum accumulation stays f32.

vs the fp32 baseline (50.5us): PE 4x faster, DMA bytes halved
(12.6 MB -> 6.4 MB/core), elementwise mults in DVE 2x mode, and the
PSUM->SBUF conversion copies split across ACT (10) and Pool (6) at
[128,1024] (2-psum-bank) granularity. DMAs are batched: 4 input chunks
of 512 KB, 8 output chunks of 512 KB, to keep SP sequencer time
(~565ns/dma_start) and per-partition lines (>=2KB) efficient.
"""

import json
import numpy as np
import ml_dtypes

BF16 = ml_dtypes.bfloat16

# ---------------------------------------------------------------- problem
B = 4096
DIM = 64
NCORES = 8
BPC = B // NCORES  # 512 batch rows per core
LMAX = 3
NMULT = 4  # multiplicity of each l in '4x0e+4x1o+4x2e+4x3o'
LS = [l for l in range(LMAX + 1) for _ in range(NMULT)]

# block-diagonal packing of the 16 (l1,l2) pair matrices into 2 stationaries
PAIRS_A = [(3, 3), (3, 2), (2, 3), (1, 1)]
PAIRS_B = [(2, 2), (1, 3), (3, 1), (1, 2), (2, 1), (0, 3), (3, 0),
           (0, 2), (2, 0), (0, 1), (1, 0), (0, 0)]

# copy index c = (s*4+u)*2 + vp (0..15): which go to the DVE engine
# (GPSIMD/Pool cannot access PSUM; ACT takes the rest)
DVE_COPIES = frozenset({4, 9, 14})

_decomp_cache = None
_nc_cache = None


def _col_start(l, u):
    return sum((2 * ll + 1) * NMULT for ll in range(l)) + u * (2 * l + 1)


def _build_decomp():
    """Index bookkeeping only (no numerics): which cb entries form the two
    stationary matrices, which in1/in2 columns feed each partition row,
    and which output row h each psum row maps to."""
    global _decomp_cache
    if _decomp_cache is not None:
        return _decomp_cache

    # replicate build_cb_matrix's row layout
    layout = {}
    idx1 = 0
    for l1 in LS:
        idx2 = 0
        for l2 in LS:
            for l3 in range(abs(l1 - l2), l1 + l2 + 1):
                layout.setdefault(l3, []).append((l1, l2, idx1 * DIM + idx2))
            idx2 += 2 * l2 + 1
        idx1 += 2 * l1 + 1
    entry_row = {}
    row = 0
    for l3 in sorted(layout):
        for (l1, l2, co) in sorted(layout[l3], key=lambda x: x[0] * LMAX + x[1]):
            entry_row[(l3, co)] = row
            row += 2 * l3 + 1
    assert row == B

    groups = []
    for pairs in (PAIRS_A, PAIRS_B):
        assert sum((2 * a + 1) * (2 * b + 1) for a, b in pairs) == 128
        c1 = np.zeros((NMULT, 128), dtype=np.int64)
        c2 = np.zeros((NMULT, 128), dtype=np.int64)
        h_of = np.zeros((NMULT, NMULT, 128), dtype=np.int64)
        w_k, w_m, w_h, w_c = [], [], [], []  # W[k,m] = cb[h, c]
        off = 0
        for (l1, l2) in pairs:
            n1, n2 = 2 * l1 + 1, 2 * l2 + 1
            kp = n1 * n2
            kk = np.arange(kp)
            m1, m2 = kk // n2, kk % n2
            for u in range(NMULT):
                c1[u, off:off + kp] = _col_start(l1, u) + m1
            for v in range(NMULT):
                c2[v, off:off + kp] = _col_start(l2, v) + m2
            mm = 0
            for l3 in range(abs(l1 - l2), l1 + l2 + 1):
                n3 = 2 * l3 + 1
                h0 = entry_row[(l3, _col_start(l1, 0) * DIM + _col_start(l2, 0))]
                km, m3m = np.meshgrid(kk, np.arange(n3), indexing="ij")
                w_k.append((off + km).ravel())
                w_m.append((off + mm + m3m).ravel())
                w_h.append((h0 + m3m).ravel())
                w_c.append(((_col_start(l1, 0) + m1[km.ravel()]) * DIM
                            + (_col_start(l2, 0) + m2[km.ravel()])))
                for u in range(NMULT):
                    for v in range(NMULT):
                        h = entry_row[(l3, _col_start(l1, u) * DIM + _col_start(l2, v))]
                        h_of[u, v, off + mm:off + mm + n3] = np.arange(h, h + n3)
                mm += n3
            off += kp
        groups.append({
            "c1": c1, "c2": c2, "h_of": h_of,
            "w_k": np.concatenate(w_k), "w_m": np.concatenate(w_m),
            "w_h": np.concatenate(w_h), "w_c": np.concatenate(w_c),
        })

    # global output row -> h map: tile t = S*16 + u*4 + v holds rows
    # t*128 + mm  ->  h_of[S][u, v, mm]
    hglob = np.zeros(32 * 128, dtype=np.int64)
    for s, g in enumerate(groups):
        for u in range(NMULT):
            for v in range(NMULT):
                t = s * 16 + u * 4 + v
                hglob[t * 128:(t + 1) * 128] = g["h_of"][u, v]
    _decomp_cache = (groups, hglob)
    return _decomp_cache


def _split_waits(bir_bytes):
    """This container's walrus build rejects >1 sync-wait per instruction
    ("Too many sync wait commands"). Hoist extra waits onto standalone
    EventSemaphore instructions on the same engine (same lowering raw
    bass wait_ge uses)."""
    bir = json.loads(bir_bytes)
    n = 0
    for fn in bir["functions"]:
        for blk in fn["blocks"]:
            out = []
            for inst in blk["instructions"]:
                si = inst.get("sync_info")
                waits = (si or {}).get("on_wait") or []
                if len(waits) > 1:
                    for w in waits[:-1]:
                        n += 1
                        out.append({
                            "debug": inst.get("debug", 0),
                            "engine": inst["engine"],
                            "ins": [], "outs": [],
                            "name": f"I-wsplit-{n}",
                            "opcode": "EventSemaphore",
                            "sync_info": {"on_update": [], "on_wait": [w]},
                        })
                    si["on_wait"] = [waits[-1]]
                out.append(inst)
            blk["instructions"] = out
    return json.dumps(bir).encode()


def _build_nc():
    """Bass program, identical on all 8 cores (SPMD; per-core data differs).

    Inputs (per core): w [128, 256] bf16 (two 128x128 stationaries),
    g [4, 128, 2048] bf16 (chunks: g1A(u0..3), g2A(v0..3), g1B, g2B,
    each chunk 4 pre-gathered [128,512] tiles along free dim).
    Output: o [128, 16384] bf16; chunk m = s*4+u at m*2048, holding
    tiles (s,u,v=0..3) as v*512+f.

    Per (s,u): one DVE mult [128, 4, 512] (g1 tile broadcast over v) ->
    rhs [128, 2048]; 4 matmuls into two 2-bank psum tiles; 2 conversion
    copies (ACT or Pool per POOL_COPIES); one 512KB output DMA.
    """
    global _nc_cache
    if _nc_cache is not None:
        return _nc_cache
    import concourse.bass as bass
    import concourse.mybir as mybir
    from concourse.tile import TileContext

    f32 = mybir.dt.float32
    bf16 = mybir.dt.bfloat16
    nc = bass.Bass()
    w = nc.dram_tensor("w", [128, 256], bf16, kind="ExternalInput")
    g = nc.dram_tensor("g", [4, 128, 2048], bf16, kind="ExternalInput")
    o = nc.dram_tensor("o", [128, 16384], bf16, kind="ExternalOutput")

    with TileContext(nc) as tc:
        with (
            tc.tile_pool(name="wpool", bufs=1) as wpool,
            tc.tile_pool(name="gpool", bufs=1) as gpool,
            tc.tile_pool(name="rhspool", bufs=3) as rhspool,
            tc.tile_pool(name="psum", bufs=4, space="PSUM") as psumpool,
            tc.tile_pool(name="opool", bufs=4) as opool,
        ):
            wt = wpool.tile([128, 256], bf16, tag="w", name="w")
            nc.sync.dma_start(out=wt, in_=w[:, :])
            gt = []
            for i in range(4):
                t = gpool.tile([128, 2048], bf16, tag=f"g{i}", name=f"g{i}")
                nc.sync.dma_start(out=t, in_=g[i])
                gt.append(t)

            for s in range(2):
                g1c, g2c = gt[2 * s], gt[2 * s + 1]
                for u in range(NMULT):
                    m = s * 4 + u
                    rhs = rhspool.tile([128, 2048], bf16, tag="rhs")
                    nc.vector.tensor_mul(
                        out=rhs[:, :].rearrange("p (v f) -> p v f", v=4),
                        in0=g1c[:, u * 512:(u + 1) * 512]
                            .unsqueeze(1).broadcast_to((128, 4, 512)),
                        in1=g2c[:, :].rearrange("p (v f) -> p v f", v=4))
                    ot = opool.tile([128, 2048], bf16, tag="ot")
                    for vp in range(2):
                        ps = psumpool.tile([128, 1024], f32, tag="ps")
                        for j in range(2):
                            v = 2 * vp + j
                            nc.tensor.matmul(
                                ps[:, j * 512:(j + 1) * 512],
                                wt[:, s * 128:(s + 1) * 128],
                                rhs[:, v * 512:(v + 1) * 512],
                                start=True, stop=True)
                        c = m * 2 + vp
                        dst = ot[:, vp * 1024:(vp + 1) * 1024]
                        if c in DVE_COPIES:
                            nc.vector.tensor_copy(out=dst, in_=ps[:, :])
                        else:
                            nc.scalar.copy(out=dst, in_=ps[:, :])
                    nc.sync.dma_start(
                        out=o[:, m * 2048:(m + 1) * 2048], in_=ot)

    orig = nc.to_json_bytes
    nc.to_json_bytes = lambda: _split_waits(orig())
    _nc_cache = nc
    return nc


def kernel(in1, in2, cb, _want_stats=False):
    from concourse.bass_utils import run_bass_kernel_spmd

    in1 = np.ascontiguousarray(np.asarray(in1, dtype=np.float32))
    in2 = np.ascontiguousarray(np.asarray(in2, dtype=np.float32))
    cb = np.asarray(cb, dtype=np.float32)
    groups, hglob = _build_decomp()

    # stationaries extracted straight from cb (no wigner math needed)
    wmat = np.zeros((128, 256), dtype=np.float32)
    for s, g in enumerate(groups):
        wmat[:, s * 128:(s + 1) * 128][g["w_k"], g["w_m"]] = cb[g["w_h"], g["w_c"]]
    wmat = wmat.astype(BF16)

    in1b = in1.astype(BF16)
    in2b = in2.astype(BF16)

    in_maps = []
    for c in range(NCORES):
        sl = slice(c * BPC, (c + 1) * BPC)
        b1t = np.ascontiguousarray(in1b[sl].T)  # [64, 512]
        b2t = np.ascontiguousarray(in2b[sl].T)
        gg = np.empty((4, 128, 2048), dtype=BF16)
        for s, grp in enumerate(groups):
            # chunk[p, u*512+f] = bT[c[u, p], f]
            gg[2 * s] = b1t[grp["c1"]].transpose(1, 0, 2).reshape(128, 2048)
            gg[2 * s + 1] = b2t[grp["c2"]].transpose(1, 0, 2).reshape(128, 2048)
        in_maps.append({"w": wmat, "g": gg})

    nc = _build_nc()
    import os
    trace = bool(int(os.environ.get("KERNEL_TRACE", "0")))
    res = run_bass_kernel_spmd(nc, in_maps, core_ids=list(range(NCORES)),
                               trace=trace)

    # o [128, 16384] -> [128, 8(m), 4(v), 512] -> rows (m,v) order = tile t
    full = np.concatenate(
        [np.asarray(r["o"]).reshape(128, 8, 4, 512)
         .transpose(1, 2, 0, 3).reshape(4096, BPC)
         for r in res.results], axis=1)
    out = np.empty((B, B), dtype=np.float32)
    out[:, hglob] = full.T.astype(np.float32)
    if _want_stats:
        return out, res
    return out


if __name__ == "__main__":
    rng = np.random.default_rng(0)
    a = rng.standard_normal((B, DIM)).astype(np.float32)
    b = rng.standard_normal((B, DIM)).astype(np.float32)
    cb = np.load("/tmp/cb.npy")
    out = kernel(a, b, cb)
    outer = np.einsum("bi,bj->bij", a, b).reshape(B, -1)
    exp = outer @ cb.T
    print("rel err:", np.linalg.norm(out - exp) / np.linalg.norm(exp))


# revision 7
# speedup vs baseline: 1.5605x; 1.1908x over previous
"""COOTensorProduct kernel for 8 Trainium2 NeuronCores — bf16 edition.

Math: out[b, h] = sum_{i,j} cb[h, i*64+j] * in1[b, i] * in2[b, j]
with in1/in2 [4096, 64], cb [4096, 4096] (Clebsch-Gordan coupling for
irreps '4x0e+4x1o+4x2e+4x3o' x same -> all l3).

cb is block-structured: the 16 (l1,l2) pair couplings pack into two
128x128 block-diagonal stationaries. Per core (512 batch rows):
  rhs[s,u,v] = g1[s,u] * g2[s,v]            (DVE, bf16, 2x mode)
  psum[s,u,v] = W_s.T @ rhs                 (PE, bf16, 1 cyc/row)
  out tile = bf16(psum)                     (ACT/Pool copies)
The rel-err budget (2e-2) dwarfs bf16 rounding (~0.5%), so all traffic
and compute run in bf16; psum accumulation stays f32.

vs the fp32 baseline (50.5us): PE 4x faster, DMA bytes halved
(12.6 MB -> 6.4 MB/core), elementwise mults in DVE 2x mode, and the
PSUM->SBUF conversion copies split across ACT (10) and Pool (6) at
[128,1024] (2-psum-bank) granularity. DMAs are batched: 4 input chunks
of 512 KB, 8 output chunks of 512 KB, to keep SP sequencer time
(~565ns/dma_start) and per-partition lines (>=2KB) efficient.
"""

import json
import numpy as np
import ml_dtypes

BF16 = ml_dtypes.bfloat16

# ---------------------------------------------------------------- problem
B = 4096
DIM = 64
NCORES = 8
BPC = B // NCORES  # 512 batch rows per core
LMAX = 3
NMULT = 4  # multiplicity of each l in '4x0e+4x1o+4x2e+4x3o'
LS = [l for l in range(LMAX + 1) for _ in range(NMULT)]

# block-diagonal packing of the 16 (l1,l2) pair matrices into 2 stationaries
PAIRS_A = [(3, 3), (3, 2), (2, 3), (1, 1)]
PAIRS_B = [(2, 2), (1, 3), (3, 1), (1, 2), (2, 1), (0, 3), (3, 0),
           (0, 2), (2, 0), (0, 1), (1, 0), (0, 0)]

# copy index c = (s*4+u)*2 + vp (0..15): which go to the DVE engine
# (GPSIMD/Pool cannot access PSUM; ACT takes the rest)
DVE_COPIES = frozenset({3, 7, 11, 15})

_decomp_cache = None
_nc_cache = None


def _col_start(l, u):
    return sum((2 * ll + 1) * NMULT for ll in range(l)) + u * (2 * l + 1)


def _build_decomp():
    """Index bookkeeping only (no numerics): which cb entries form the two
    stationary matrices, which in1/in2 columns feed each partition row,
    and which output row h each psum row maps to."""
    global _decomp_cache
    if _decomp_cache is not None:
        return _decomp_cache

    # replicate build_cb_matrix's row layout
    layout = {}
    idx1 = 0
    for l1 in LS:
        idx2 = 0
        for l2 in LS:
            for l3 in range(abs(l1 - l2), l1 + l2 + 1):
                layout.setdefault(l3, []).append((l1, l2, idx1 * DIM + idx2))
            idx2 += 2 * l2 + 1
        idx1 += 2 * l1 + 1
    entry_row = {}
    row = 0
    for l3 in sorted(layout):
        for (l1, l2, co) in sorted(layout[l3], key=lambda x: x[0] * LMAX + x[1]):
            entry_row[(l3, co)] = row
            row += 2 * l3 + 1
    assert row == B

    groups = []
    for pairs in (PAIRS_A, PAIRS_B):
        assert sum((2 * a + 1) * (2 * b + 1) for a, b in pairs) == 128
        c1 = np.zeros((NMULT, 128), dtype=np.int64)
        c2 = np.zeros((NMULT, 128), dtype=np.int64)
        h_of = np.zeros((NMULT, NMULT, 128), dtype=np.int64)
        w_k, w_m, w_h, w_c = [], [], [], []  # W[k,m] = cb[h, c]
        off = 0
        for (l1, l2) in pairs:
            n1, n2 = 2 * l1 + 1, 2 * l2 + 1
            kp = n1 * n2
            kk = np.arange(kp)
            m1, m2 = kk // n2, kk % n2
            for u in range(NMULT):
                c1[u, off:off + kp] = _col_start(l1, u) + m1
            for v in range(NMULT):
                c2[v, off:off + kp] = _col_start(l2, v) + m2
            mm = 0
            for l3 in range(abs(l1 - l2), l1 + l2 + 1):
                n3 = 2 * l3 + 1
                h0 = entry_row[(l3, _col_start(l1, 0) * DIM + _col_start(l2, 0))]
                km, m3m = np.meshgrid(kk, np.arange(n3), indexing="ij")
                w_k.append((off + km).ravel())
                w_m.append((off + mm + m3m).ravel())
                w_h.append((h0 + m3m).ravel())
                w_c.append(((_col_start(l1, 0) + m1[km.ravel()]) * DIM
                            + (_col_start(l2, 0) + m2[km.ravel()])))
                for u in range(NMULT):
                    for v in range(NMULT):
                        h = entry_row[(l3, _col_start(l1, u) * DIM + _col_start(l2, v))]
                        h_of[u, v, off + mm:off + mm + n3] = np.arange(h, h + n3)
                mm += n3
            off += kp
        groups.append({
            "c1": c1, "c2": c2, "h_of": h_of,
            "w_k": np.concatenate(w_k), "w_m": np.concatenate(w_m),
            "w_h": np.concatenate(w_h), "w_c": np.concatenate(w_c),
        })

    # global output row -> h map: tile t = S*16 + u*4 + v holds rows
    # t*128 + mm  ->  h_of[S][u, v, mm]
    hglob = np.zeros(32 * 128, dtype=np.int64)
    for s, g in enumerate(groups):
        for u in range(NMULT):
            for v in range(NMULT):
                t = s * 16 + u * 4 + v
                hglob[t * 128:(t + 1) * 128] = g["h_of"][u, v]
    _decomp_cache = (groups, hglob)
    return _decomp_cache


def _split_waits(bir_bytes):
    """This container's walrus build rejects >1 sync-wait per instruction
    ("Too many sync wait commands"). Hoist extra waits onto standalone
    EventSemaphore instructions on the same engine (same lowering raw
    bass wait_ge uses)."""
    bir = json.loads(bir_bytes)
    n = 0
    for fn in bir["functions"]:
        for blk in fn["blocks"]:
            out = []
            for inst in blk["instructions"]:
                si = inst.get("sync_info")
                waits = (si or {}).get("on_wait") or []
                if len(waits) > 1:
                    for w in waits[:-1]:
                        n += 1
                        out.append({
                            "debug": inst.get("debug", 0),
                            "engine": inst["engine"],
                            "ins": [], "outs": [],
                            "name": f"I-wsplit-{n}",
                            "opcode": "EventSemaphore",
                            "sync_info": {"on_update": [], "on_wait": [w]},
                        })
                    si["on_wait"] = [waits[-1]]
                out.append(inst)
            blk["instructions"] = out
    return json.dumps(bir).encode()


def _build_nc():
    """Bass program, identical on all 8 cores (SPMD; per-core data differs).

    Inputs (per core): w [128, 256] bf16 (two 128x128 stationaries),
    g [4, 128, 2048] bf16 (chunks: g1A(u0..3), g2A(v0..3), g1B, g2B,
    each chunk 4 pre-gathered [128,512] tiles along free dim).
    Output: o [128, 16384] bf16; chunk m = s*4+u at m*2048, holding
    tiles (s,u,v=0..3) as v*512+f.

    Per (s,u): one DVE mult [128, 4, 512] (g1 tile broadcast over v) ->
    rhs [128, 2048]; 4 matmuls into two 2-bank psum tiles; 2 conversion
    copies (ACT or Pool per POOL_COPIES); one 512KB output DMA.
    """
    global _nc_cache
    if _nc_cache is not None:
        return _nc_cache
    import concourse.bass as bass
    import concourse.mybir as mybir
    from concourse.tile import TileContext

    f32 = mybir.dt.float32
    bf16 = mybir.dt.bfloat16
    nc = bass.Bass()
    w = nc.dram_tensor("w", [128, 256], bf16, kind="ExternalInput")
    g = nc.dram_tensor("g", [4, 128, 2048], bf16, kind="ExternalInput")
    o = nc.dram_tensor("o", [128, 16384], bf16, kind="ExternalOutput")

    with TileContext(nc) as tc:
        with (
            tc.tile_pool(name="wpool", bufs=1) as wpool,
            tc.tile_pool(name="gpool", bufs=1) as gpool,
            tc.tile_pool(name="rhspool", bufs=2) as rhspool,
            tc.tile_pool(name="psum", bufs=4, space="PSUM") as psumpool,
            tc.tile_pool(name="opool", bufs=4) as opool,
        ):
            wt = wpool.tile([128, 256], bf16, tag="w", name="w")
            nc.sync.dma_start(out=wt, in_=w[:, :])
            # input order: g2 chunk first, then g1 in u01/u23 halves, so the
            # first mult (needs all of g2s + half of g1s) starts ~4us sooner
            gt = {}
            for s in range(2):
                t = gpool.tile([128, 2048], bf16, tag=f"g2_{s}", name=f"g2_{s}")
                nc.sync.dma_start(out=t, in_=g[2 * s + 1])
                gt[(2, s)] = t
                t = gpool.tile([128, 2048], bf16, tag=f"g1_{s}", name=f"g1_{s}")
                for h in range(2):
                    nc.sync.dma_start(
                        out=t[:, h * 1024:(h + 1) * 1024],
                        in_=g[2 * s][:, h * 1024:(h + 1) * 1024])
                gt[(1, s)] = t

            for s in range(2):
                g1c, g2c = gt[(1, s)], gt[(2, s)]
                for up in range(2):  # u pair: covers u = 2*up, 2*up+1
                    rhs = rhspool.tile([128, 4096], bf16, tag="rhs")
                    nc.vector.tensor_mul(
                        out=rhs[:, :].rearrange(
                            "p (u v f) -> p u v f", u=2, v=4),
                        in0=g1c[:, up * 1024:(up + 1) * 1024]
                            .rearrange("p (u f) -> p u f", u=2)
                            .unsqueeze(2).broadcast_to((128, 2, 4, 512)),
                        in1=g2c[:, :].rearrange("p (v f) -> p v f", v=4)
                            .unsqueeze(1).broadcast_to((128, 2, 4, 512)))
                    for du in range(2):
                        u = 2 * up + du
                        m = s * 4 + u
                        ot = opool.tile([128, 2048], bf16, tag="ot")
                        for vp in range(2):
                            ps = psumpool.tile([128, 1024], f32, tag="ps")
                            for j in range(2):
                                v = 2 * vp + j
                                nc.tensor.matmul(
                                    ps[:, j * 512:(j + 1) * 512],
                                    wt[:, s * 128:(s + 1) * 128],
                                    rhs[:, du * 2048 + v * 512:
                                        du * 2048 + (v + 1) * 512],
                                    start=True, stop=True)
                            c = m * 2 + vp
                            dst = ot[:, vp * 1024:(vp + 1) * 1024]
                            if c in DVE_COPIES:
                                nc.vector.tensor_copy(out=dst, in_=ps[:, :])
                            else:
                                nc.scalar.copy(out=dst, in_=ps[:, :])
                        nc.sync.dma_start(
                            out=o[:, m * 2048:(m + 1) * 2048], in_=ot)

    orig = nc.to_json_bytes
    nc.to_json_bytes = lambda: _split_waits(orig())
    _nc_cache = nc
    return nc


def kernel(in1, in2, cb, _want_stats=False):
    from concourse.bass_utils import run_bass_kernel_spmd

    in1 = np.ascontiguousarray(np.asarray(in1, dtype=np.float32))
    in2 = np.ascontiguousarray(np.asarray(in2, dtype=np.float32))
    cb = np.asarray(cb, dtype=np.float32)
    groups, hglob = _build_decomp()

    # stationaries extracted straight from cb (no wigner math needed)
    wmat = np.zeros((128, 256), dtype=np.float32)
    for s, g in enumerate(groups):
        wmat[:, s * 128:(s + 1) * 128][g["w_k"], g["w_m"]] = cb[g["w_h"], g["w_c"]]
    wmat = wmat.astype(BF16)

    in1b = in1.astype(BF16)
    in2b = in2.astype(BF16)

    in_maps = []
    for c in range(NCORES):
        sl = slice(c * BPC, (c + 1) * BPC)
        b1t = np.ascontiguousarray(in1b[sl].T)  # [64, 512]
        b2t = np.ascontiguousarray(in2b[sl].T)
        gg = np.empty((4, 128, 2048), dtype=BF16)
        for s, grp in enumerate(groups):
            # chunk[p, u*512+f] = bT[c[u, p], f]
            gg[2 * s] = b1t[grp["c1"]].transpose(1, 0, 2).reshape(128, 2048)
            gg[2 * s + 1] = b2t[grp["c2"]].transpose(1, 0, 2).reshape(128, 2048)
        in_maps.append({"w": wmat, "g": gg})

    nc = _build_nc()
    import os
    trace = bool(int(os.environ.get("KERNEL_TRACE", "0")))
    res = run_bass_kernel_spmd(nc, in_maps, core_ids=list(range(NCORES)),
                               trace=trace)

    # o [128, 16384] -> [128, 8(m), 4(v), 512] -> rows (m,v) order = tile t
    full = np.concatenate(
        [np.asarray(r["o"]).reshape(128, 8, 4, 512)
         .transpose(1, 2, 0, 3).reshape(4096, BPC)
         for r in res.results], axis=1)
    out = np.empty((B, B), dtype=np.float32)
    out[:, hglob] = full.T.astype(np.float32)
    if _want_stats:
        return out, res
    return out


if __name__ == "__main__":
    rng = np.random.default_rng(0)
    a = rng.standard_normal((B, DIM)).astype(np.float32)
    b = rng.standard_normal((B, DIM)).astype(np.float32)
    cb = np.load("/tmp/cb.npy")
    out = kernel(a, b, cb)
    outer = np.einsum("bi,bj->bij", a, b).reshape(B, -1)
    exp = outer @ cb.T
    print("rel err:", np.linalg.norm(out - exp) / np.linalg.norm(exp))
